# revision 31
# baseline (speedup 1.0000x reference)
"""AttnBlock fusion kernel for Trainium2 (Bass/Tile), 8 NeuronCores.

Reference computation (per batch element b; c=512 channels, hw=1024 spatial):
    h  = GroupNorm(32, c)(x) ; k = Wk h + bk ; v = Wv h + bv
    y_ = GroupNorm(32, c)(y) ; q = Wq y_ + bq
    attn = softmax_j(q^T k / sqrt(c)) ; o = v @ attn^T ; out = x + Wp o + bp

Sharding: pure data parallel over batch (16 batches / 8 cores = 2 each).

Algebraic folds (host side, exact):
  * S = q^T k = y_^T (Wq^T Wk) h  -> A := Wq^T Wk precomputed; the q and k
    projections disappear (one matmul t = A h replaces both).
  * Wp (v @ P) = (Wp Wv) h @ P    -> Bm := Wp Wv precomputed; the v and
    proj_out projections disappear (u = Bm h replaces both).
  * bk adds a per-i constant to logits -> cancels in softmax.
  * bv contributes Wp bv exactly (softmax rows sum to 1) -> bp' = bp + Wp bv.
  * bq (zero in practice) handled by a compiled-in logit-bias path.

Precision scheme (validated vs reference: rel_l2 ~ 5.6e-3, gate 2e-2):
  * All five big matmuls run fp8(e4m3) with MatmulPerfMode.DoubleRow:
    2 contraction tiles per pass = 2x throughput over fp32r/bf16.
  * A, Bm scaled by 16 so t = A h and u = Bm h land in e4m3 range (+-240);
    1/16 is folded into the exp scale (t side) and into ones=16 for the
    Z row-sum matmul (u side, via 1/Z).
  * E = exp(s S - 3): the -3 shift is softmax-invariant and keeps
    max(E) ~ 31 < 240 so no fp8 overflow-to-inf.
  * x, y stored bf16 on chip (stats + gn + residual), accumulation fp32.

Engine split per batch (PE ~28us is the bound):
  PE      t/uT/S/Z/o DoubleRow matmuls + tiny GN aggregation matmul
  Scalar  exp(S), t PSUM->fp8 copies, GN sqrt
  DVE     bn_stats GN statistics, uT copies, 1/Z (fast approx), o*(1/Z)
  GpSimd  GN apply (bf16->fp8), residual add to output
"""

import math
import os
import sys
from contextlib import ExitStack

import numpy as np
import ml_dtypes

for _p in ("/opt/trn_rl_repo", "/root/.axon_site/_ro/trn_rl_repo"):
    if os.path.isdir(_p) and _p not in sys.path:
        sys.path.append(_p)

import concourse.bass as bass
import concourse.bacc as bacc
import concourse.mybir as mybir
import concourse.tile as tile
from concourse.bass_utils import run_bass_kernel_spmd

F32 = mybir.dt.float32
BF16 = mybir.dt.bfloat16
F8 = mybir.dt.float8e4
U8 = mybir.dt.uint8
U16 = mybir.dt.uint16
AF = mybir.ActivationFunctionType
ALU = mybir.AluOpType
DR = mybir.MatmulPerfMode.DoubleRow

B, C, H, W = 16, 512, 32, 32
HW = H * W                  # 1024
NCORES = 8
BPC = B // NCORES           # 2 batches per core
P = 128                     # SBUF partitions
CT = C // P                 # 4 channel tiles
JT = HW // P                # 8 key-position tiles
IBS = 512                   # query positions per i-block
IB = HW // IBS              # 2 i-blocks
GROUPS = 32
GSIZE = C // GROUPS         # 16 channels per group
EPS = 1e-6
SM = float(C) ** -0.5
SA = 16.0                   # scale folded into A
SB = 16.0                   # scale folded into Bm (and into ones for Z)
EXPS = SM / SA
EXPB = -3.0                 # softmax-invariant logit shift, keeps E < 240

NPF8 = ml_dtypes.float8_e4m3   # IEEE e4m3 (bias 7, max 240) == TRN FP8_EXP4
NPBF16 = ml_dtypes.bfloat16


def _emit(tc, aps, has_bq, has_bpp, id_aff):
    nc = tc.nc
    xs, ys, out = aps["xs"], aps["ys"], aps["out"]

    with ExitStack() as ctx:
        cpool = ctx.enter_context(tc.tile_pool(name="const", bufs=1))
        wpool = ctx.enter_context(tc.tile_pool(name="w", bufs=1))
        xpool = ctx.enter_context(tc.tile_pool(name="xin", bufs=2))
        ypool = ctx.enter_context(tc.tile_pool(name="yin", bufs=2))
        hpool = ctx.enter_context(tc.tile_pool(name="hb", bufs=2))
        ynpool = ctx.enter_context(tc.tile_pool(name="ynb", bufs=2))
        tpool = ctx.enter_context(tc.tile_pool(name="tb", bufs=2))
        upool = ctx.enter_context(tc.tile_pool(name="ub", bufs=2))
        epool = ctx.enter_context(tc.tile_pool(name="eb", bufs=2))
        stpool = ctx.enter_context(tc.tile_pool(name="st", bufs=2))
        smpool = ctx.enter_context(tc.tile_pool(name="sm", bufs=3))
        ompool = ctx.enter_context(tc.tile_pool(name="om", bufs=3))
        outpool = ctx.enter_context(tc.tile_pool(name="outb", bufs=2))
        rzpool = ctx.enter_context(tc.tile_pool(name="rz", bufs=2))
        # one 7-bank ring for S/t/uT tiles AND o accumulators: during the
        # S phase all 7 banks buffer S tiles so the PE can run far ahead of
        # the exp drain; o accumulators then claim 4 slots by rotation
        pspool = ctx.enter_context(tc.tile_pool(name="ps", bufs=7, space="PSUM"))
        zpool = ctx.enter_context(tc.tile_pool(name="z", bufs=1, space="PSUM"))
        opool = pspool

        # ---- constants / weights ----
        prm_sb = cpool.tile([P, 5, CT], F32)
        nc.gpsimd.dma_start(prm_sb[:], aps["prm"].rearrange("p (q t) -> p q t", t=CT))
        amat_sb = cpool.tile([P, P], F32)
        nc.gpsimd.dma_start(amat_sb[:], aps["amat"][:])
        ones_sb = cpool.tile([P, 2, P], F8)
        nc.vector.memset(ones_sb[:], SB)
        expb_sb = cpool.tile([P, 1], F32)
        nc.vector.memset(expb_sb[:], EXPB)
        eps_sb = cpool.tile([P, 1], F32)
        nc.vector.memset(eps_sb[:], EPS)
        # dummy matmul to absorb the PE's cold-start latency before the
        # first real (dependency-gated) matmul arrives
        wps = pspool.tile([P, P], F32, tag="ps", name="warm")
        nc.tensor.matmul(wps[:], ones_sb[:], ones_sb[:], start=True, stop=True,
                         perf_mode=DR)
        A_sb = wpool.tile([P, 2, 2, C], F8)
        nc.gpsimd.dma_start(
            A_sb[:], aps["A"].rearrange("p (a b o) -> p a b o", a=2, b=2).bitcast(F8)
        )
        Bm_sb = wpool.tile([P, 2, 2, C], F8)
        nc.gpsimd.dma_start(
            Bm_sb[:], aps["Bm"].rearrange("p (a b o) -> p a b o", a=2, b=2).bitcast(F8)
        )
        if has_bq:
            g_sb = cpool.tile([P, CT], F8)
            nc.gpsimd.dma_start(g_sb[:], aps["gv"].bitcast(F8))

        def load_xy(b):
            """Split DMAs so stats can start before the full tensor lands;
            x0 per-tile since its first tile gates the whole pipeline."""
            x_sb = xpool.tile([P, CT, HW], BF16, tag="x")
            v = xs[b].rearrange("p (t n) -> p t n", n=HW).bitcast(BF16)
            step = 1 if b == 0 else 2
            for c0 in range(0, CT, step):
                nc.sync.dma_start(x_sb[:, c0 : c0 + step, :],
                                  v[:, c0 : c0 + step, :])
            y_sb = ypool.tile([P, CT, HW], BF16, tag="y")
            v = ys[b].rearrange("p (t n) -> p t n", n=HW).bitcast(BF16)
            for hf in range(2):
                nc.sync.dma_start(y_sb[:, 2 * hf : 2 * hf + 2, :],
                                  v[:, 2 * hf : 2 * hf + 2, :])
            return x_sb, y_sb

        def stats_pre(src, st, u):
            """DVE bn_stats: per-channel [mean, var, mean^2] -> st[:, u]."""
            for t in range(CT):
                bns = smpool.tile([P, 2, 6], F32, tag="bns")
                for h2 in range(2):
                    nc.vector.bn_stats(
                        bns[:, h2, :], src[:, t, h2 * 512 : (h2 + 1) * 512]
                    )
                nc.vector.bn_aggr(st[:, u, t, 0:2], bns[:])
            nc.vector.tensor_tensor(
                st[:, u, :, 2], st[:, u, :, 0], st[:, u, :, 0], op=ALU.mult
            )

        def stats_mm(st, u):
            """PE: per-group averaging of [mean, var, mean^2] for one tensor."""
            gt = pspool.tile([P, IBS], F32, tag="ps", name="gps")
            gps = gt[:, 0 : CT * 3]
            nc.tensor.matmul(gps, amat_sb[:], st[:, u], start=True, stop=True)
            return gps

        def stats_post(gps, u, uid):
            """a = rstd*gamma, mb = beta - mean*a for one tensor: [P, CT].
            rstd = 1/(sqrt on ACT, accurate reciprocal on DVE); chain kept
            short because each serial hop pays a scheduler-interleave delay."""
            g = smpool.tile([P, CT, 3], F32, tag=f"g{uid}")
            nc.vector.tensor_copy(g[:], gps)
            # var_g = E[var] + E[mean^2] - E[mean]^2 (equal-count partitions);
            # mid-chain ops on GpSimd so DVE backfill can't delay the hops
            msq = smpool.tile([P, CT], F32, tag=f"ms{uid}")
            nc.gpsimd.tensor_tensor(msq[:], g[:, :, 0], g[:, :, 0], op=ALU.mult)
            var = smpool.tile([P, CT], F32, tag=f"va{uid}")
            nc.gpsimd.tensor_tensor(var[:], g[:, :, 1], g[:, :, 2], op=ALU.add)
            nc.gpsimd.tensor_tensor(var[:], var[:], msq[:], op=ALU.subtract)
            std = smpool.tile([P, CT], F32, tag=f"sd{uid}")
            nc.scalar.activation(std[:], var[:], AF.Sqrt, bias=eps_sb[:])
            r0 = smpool.tile([P, CT], F32, tag=f"r0{uid}")
            nc.vector.reciprocal(r0[:], std[:])
            mb = smpool.tile([P, CT], F32, tag=f"mb{uid}")
            if id_aff:
                # gamma == 1, beta == 0: a = rstd, mb = -mean*rstd
                nc.vector.scalar_tensor_tensor(
                    mb[:], g[:, :, 0], -1.0, r0[:], op0=ALU.mult, op1=ALU.mult
                )
                return r0, mb
            a = smpool.tile([P, CT], F32, tag=f"a{uid}")
            nc.gpsimd.tensor_tensor(a[:], r0[:], prm_sb[:, u, :], op=ALU.mult)
            nc.gpsimd.tensor_tensor(mb[:], g[:, :, 0], a[:], op=ALU.mult)
            nc.gpsimd.tensor_tensor(mb[:], prm_sb[:, 2 + u, :], mb[:], op=ALU.subtract)
            return a, mb

        def gn_apply(src, pool, tag, ab, engines):
            """Per-tile affine-normalize, tile t on engines[t] (bf16 -> fp8)."""
            a, mb = ab
            d = pool.tile([P, CT, HW], F8, tag=tag)
            for t in range(CT):
                eng = engines[t]
                if eng is nc.scalar:
                    nc.scalar.activation(
                        d[:, t, :], src[:, t, :], AF.Identity,
                        bias=mb[:, t : t + 1], scale=a[:, t : t + 1],
                    )
                else:
                    eng.tensor_scalar(
                        d[:, t, :], src[:, t, :], a[:, t : t + 1],
                        mb[:, t : t + 1], op0=ALU.mult, op1=ALU.add,
                    )
            return d

        V, G, SC = nc.vector, nc.gpsimd, nc.scalar

        def emit_t(h_sb):
            """t = A h  (t[cy, j], fp8; copies on Scalar)."""
            t_sb = tpool.tile([P, CT, HW], F8, tag="t", name="t")
            for nh in range(IB):
                for mt in range(CT):
                    ps = pspool.tile([P, IBS], F32, tag="ps", name="ps")
                    for kp in range(2):
                        nc.tensor.matmul(
                            ps[:],
                            A_sb[:, kp, :, mt * P : (mt + 1) * P],
                            h_sb[:, 2 * kp : 2 * kp + 2, nh * IBS : (nh + 1) * IBS],
                            start=(kp == 0), stop=(kp == 1), perf_mode=DR,
                        )
                    nc.scalar.copy(t_sb[:, mt, nh * IBS : (nh + 1) * IBS], ps[:])
            return t_sb

        def emit_uT(h_sb):
            """uT = h^T Bm^T  (uT[j, co], fp8; copies split Scalar/DVE)."""
            uT_sb = upool.tile([P, JT, C], F8, tag="u", name="u")
            for jt in range(JT):
                ps = pspool.tile([P, C], F32, tag="ps", name="ps")
                for kp in range(2):
                    nc.tensor.matmul(
                        ps[:],
                        h_sb[:, 2 * kp : 2 * kp + 2, jt * P : (jt + 1) * P],
                        Bm_sb[:, kp, :, :],
                        start=(kp == 0), stop=(kp == 1), perf_mode=DR,
                    )
                if jt < 4:
                    nc.scalar.copy(uT_sb[:, jt, :], ps[:])
                else:
                    nc.vector.tensor_copy(uT_sb[:, jt, :], ps[:])
            return uT_sb

        def emit_bias(h_sb):
            """bq logit bias: r[j] = g^T h, bias = SM*r + EXPB."""
            rps = zpool.tile([P, JT], F32, tag="z", name="rb")
            for jt in range(JT):
                for kt in range(CT):
                    nc.tensor.matmul(
                        rps[:, jt : jt + 1],
                        h_sb[:, kt, jt * P : (jt + 1) * P],
                        g_sb[:, kt : kt + 1],
                        start=(kt == 0), stop=(kt == CT - 1),
                    )
            bias_sb = smpool.tile([P, JT], F32, tag="bia", name="bia")
            nc.vector.tensor_scalar(
                bias_sb[:], rps[:], SM, EXPB, op0=ALU.mult, op1=ALU.add
            )
            return bias_sb

        def emit_attention(b, t_sb, uT_sb, yn_sb, xres, bias_sb):
            outv = out[b].rearrange("p (t n) -> p t n", n=HW)
            e = [
                epool.tile([P, JT, IBS], F8, tag=f"e{ib}", name=f"e{ib}")
                for ib in range(IB)
            ]
            zps = {}

            def S_group(ib, jt):
                ps = pspool.tile([P, IBS], F32, tag="ps", name="ps")
                for kp in range(2):
                    nc.tensor.matmul(
                        ps[:],
                        t_sb[:, 2 * kp : 2 * kp + 2, jt * P : (jt + 1) * P],
                        yn_sb[:, 2 * kp : 2 * kp + 2, ib * IBS : (ib + 1) * IBS],
                        start=(kp == 0), stop=(kp == 1), perf_mode=DR,
                    )
                bias = bias_sb[:, jt : jt + 1] if has_bq else expb_sb[:]
                nc.scalar.activation(
                    e[ib][:, jt, :], ps[:], AF.Exp, bias=bias, scale=EXPS
                )

            def Z_mm(ib, pr):
                if pr == 0:
                    zps[ib] = zpool.tile([P, IBS], F32, tag="z", name="z")
                nc.tensor.matmul(
                    zps[ib][:], ones_sb[:],
                    e[ib][:, 2 * pr : 2 * pr + 2, :],
                    start=(pr == 0), stop=(pr == 3), perf_mode=DR,
                )

            def recip(ib):
                rz = rzpool.tile([P, IBS], F32, tag="rz", name="rz")
                with tc.high_priority():
                    nc.vector.reciprocal_approx_fast(rz[:], zps[ib][:])
                return rz

            def o_block(ib, rz, ot, last=False):
                isl = slice(ib * IBS, (ib + 1) * IBS)
                for ct in range(CT):
                    ops_ = opool.tile([P, IBS], F32, tag="ps", name="o")
                    for pr in range(4):
                        nc.tensor.matmul(
                            ops_[:],
                            uT_sb[:, 2 * pr : 2 * pr + 2, ct * P : (ct + 1) * P],
                            e[ib][:, 2 * pr : 2 * pr + 2, :],
                            start=(pr == 0), stop=(pr == 3), perf_mode=DR,
                        )
                    om = ompool.tile([P, IBS], F32, tag="om", name="om")
                    # high priority: PSUM drains must preempt lower-priority
                    # stats backfill in the DVE ready-queue or the o banks
                    # starve the next i-block's matmuls
                    with tc.high_priority():
                        nc.vector.tensor_tensor(om[:], ops_[:], rz[:], op=ALU.mult)
                        # final block: drain on DVE + per-ct DMA, shortest tail
                        eng = nc.vector if last else nc.gpsimd
                        eng.tensor_tensor(
                            ot[:, ct, :], om[:], xres[:, ct, isl], op=ALU.add
                        )
                        if last:
                            nc.sync.dma_start(outv[:, ct, isl], ot[:, ct, :])
                if not last:
                    nc.sync.dma_start(outv[:, :, isl], ot[:])

            ot0 = outpool.tile([P, CT, IBS], F32, tag="ot", name="ot0")
            ot1 = outpool.tile([P, CT, IBS], F32, tag="ot", name="ot1")
            for jt in range(JT):
                S_group(0, jt)
            for pr in range(3):
                Z_mm(0, pr)
            for jt in range(4):
                S_group(1, jt)
            Z_mm(0, 3)
            rz0 = recip(0)
            o_block(0, rz0, ot0)
            for jt in range(4, JT):
                S_group(1, jt)
            for pr in range(3):
                Z_mm(1, pr)
            Z_mm(1, 3)
            rz1 = recip(1)
            o_block(1, rz1, ot1, last=(b == BPC - 1))

        def make_xres(x_sb):
            if not has_bpp:
                return x_sb
            xres = outpool.tile([P, CT, HW], F32, tag="xb", name="xb")
            for t in range(CT):
                nc.gpsimd.tensor_scalar(
                    xres[:, t, :], x_sb[:, t, :], prm_sb[:, 4, t : t + 1],
                    None, op0=ALU.add,
                )
            return xres

        # ---- all GroupNorm work is front-loaded (BPC=2): the x0 chain gates
        # the first matmul; everything else fills scheduler bubbles. All
        # Sqrts precede the first Exp so the activation table loads only
        # twice in the whole kernel.
        xy = [load_xy(b) for b in range(BPC)]
        sts = [stpool.tile([P, 2, CT, 3], F32, tag="st", name=f"st{b}")
               for b in range(BPC)]

        stats_pre(xy[0][0], sts[0], 0)
        ab = stats_post(stats_mm(sts[0], 0), 0, "x0")
        h0 = gn_apply(xy[0][0], hpool, "h", ab, [V, V, SC, G])
        t0 = emit_t(h0)
        stats_pre(xy[0][1], sts[0], 1)
        ab = stats_post(stats_mm(sts[0], 1), 1, "y0")
        yn0 = gn_apply(xy[0][1], ynpool, "yn", ab, [V, SC, G, G])
        u0 = emit_uT(h0)
        bias0 = emit_bias(h0) if has_bq else None

        hs, yns, ts, us, biases = [h0], [yn0], [t0], [u0], [bias0]
        for b in range(BPC):
            xres = make_xres(xy[b][0])
            if b > 0:
                ts.append(emit_t(hs[b]))
                us.append(emit_uT(hs[b]))
                biases.append(emit_bias(hs[b]) if has_bq else None)
            emit_attention(b, ts[b], us[b], yns[b], xres, biases[b])
            if b + 1 < BPC:
                # next batch's GN emitted AFTER this attention: lower priority,
                # so the scheduler backfills engine-idle slots with it instead
                # of serializing this batch's exps/matmuls behind it (the PE
                # and Scalar queues are in-order)
                bn = b + 1
                stats_pre(xy[bn][0], sts[bn], 0)
                ab = stats_post(stats_mm(sts[bn], 0), 0, f"x{bn}")
                hs.append(gn_apply(xy[bn][0], hpool, "h", ab, [V, V, G, G]))
                stats_pre(xy[bn][1], sts[bn], 1)
                ab = stats_post(stats_mm(sts[bn], 1), 1, f"y{bn}")
                yns.append(gn_apply(xy[bn][1], ynpool, "yn", ab, [V, V, G, G]))


_CACHE = {}


def _build(has_bq, has_bpp, id_aff):
    key = ("nc", has_bq, has_bpp, id_aff)
    if key in _CACHE:
        return _CACHE[key]
    nc = bacc.Bacc("TRN2", target_bir_lowering=False, debug=False)
    aps = {
        "xs": nc.dram_tensor("xs", [BPC, P, CT * HW], U16, kind="ExternalInput").ap(),
        "ys": nc.dram_tensor("ys", [BPC, P, CT * HW], U16, kind="ExternalInput").ap(),
        "A": nc.dram_tensor("A", [P, 4 * C], U8, kind="ExternalInput").ap(),
        "Bm": nc.dram_tensor("Bm", [P, 4 * C], U8, kind="ExternalInput").ap(),
        "prm": nc.dram_tensor("prm", [P, 5 * CT], F32, kind="ExternalInput").ap(),
        "amat": nc.dram_tensor("amat", [P, P], F32, kind="ExternalInput").ap(),
        "out": nc.dram_tensor("out", [BPC, P, CT * HW], F32, kind="ExternalOutput").ap(),
    }
    if has_bq:
        aps["gv"] = nc.dram_tensor("gv", [P, CT], U8, kind="ExternalInput").ap()
    with tile.TileContext(nc) as tc:
        _emit(tc, aps, has_bq, has_bpp, id_aff)
    nc.compile()
    _CACHE[key] = nc
    return nc


def _pack_chw(a):
    """[*, C, HW] -> [*, P, CT*HW] matching SBUF layout c = t*128 + p."""
    lead = a.shape[:-2]
    a = a.reshape(*lead, CT, P, HW)
    a = np.moveaxis(a, -3, -2)          # [..., P, CT, HW]
    return np.ascontiguousarray(a.reshape(*lead, P, CT * HW))


def _unpack_chw(a):
    """[*, P, CT*HW] -> [*, C, HW]."""
    lead = a.shape[:-2]
    a = a.reshape(*lead, P, CT, HW)
    a = np.moveaxis(a, -2, -3)          # [..., CT, P, HW]
    return np.ascontiguousarray(a.reshape(*lead, CT * P, HW))


def _q8(a):
    return np.clip(a, -240.0, 240.0).astype(NPF8)


def _pack_w(wT, scale):
    """wT [cin, cout] -> fp8 bytes [P, 2*2*C]: [p, kpair, ktile2, cout],
    cin = (2*kpair + ktile2)*128 + p."""
    w8 = _q8(wT * scale).view(np.uint8)
    w8 = w8.reshape(2, 2, P, C).transpose(2, 0, 1, 3)
    return np.ascontiguousarray(w8.reshape(P, 4 * C))


def _host_inputs(x, y, norm_scale, norm_bias, norm1_scale, norm1_bias,
                 wq, bq, wk, bk, wv, bv, wp, bp):
    f = lambda a: np.ascontiguousarray(np.asarray(a, dtype=np.float32))
    x = f(x).reshape(B, C, HW)
    y = f(y).reshape(B, C, HW)
    wq, wk, wv, wp = f(wq), f(wk), f(wv), f(wp)
    A = wq.T @ wk                       # [cy, ch]
    Bm = wp @ wv                        # [co, ci]
    # bk cancels in softmax; bv folds into bp' because softmax rows sum to 1
    bpp = f(bp) + wp @ f(bv)
    # rows: [gamma_x, gamma_y, beta_x, beta_y, bpp]
    prm = np.stack([f(norm_scale), f(norm1_scale), f(norm_bias), f(norm1_bias),
                    bpp]).astype(np.float32)
    prm = np.ascontiguousarray(
        prm.reshape(5, CT, P).transpose(2, 0, 1).reshape(P, 5 * CT)
    )
    amat = np.zeros((P, P), np.float32)
    for g in range(P // GSIZE):
        amat[g * GSIZE : (g + 1) * GSIZE, g * GSIZE : (g + 1) * GSIZE] = 1.0 / GSIZE
    has_bq = bool(np.any(np.asarray(bq)))
    has_bpp = bool(np.any(bpp))
    id_aff = bool(
        np.all(prm[:, 0 * CT : 2 * CT] == 1.0) and
        np.all(prm[:, 2 * CT : 4 * CT] == 0.0)
    )
    shared = {
        "A": _pack_w(A.T, SA),          # lhsT[cin=ch, cout=cy]
        "Bm": _pack_w(Bm.T, SB),        # rhs[cin=ci, cout=co]
        "prm": prm, "amat": amat,
    }
    if has_bq:
        gv = wk.T @ f(bq)               # [ci]
        gv8 = _q8(gv).view(np.uint8).reshape(CT, P).T
        shared["gv"] = np.ascontiguousarray(gv8)

    xb = _pack_chw(x.astype(NPBF16).view(np.uint16))
    yb = _pack_chw(y.astype(NPBF16).view(np.uint16))
    in_maps = []
    for core in range(NCORES):
        sl = slice(core * BPC, (core + 1) * BPC)
        in_maps.append({"xs": xb[sl], "ys": yb[sl], **shared})
    return in_maps, (has_bq, has_bpp, id_aff)


def _run(in_maps, flags, trace=False):
    nc = _build(*flags)
    res = run_bass_kernel_spmd(
        nc, in_maps, core_ids=list(range(NCORES)), trace=trace
    )
    out = np.concatenate(
        [_unpack_chw(res.results[i]["out"]) for i in range(NCORES)], axis=0
    ).reshape(B, C, H, W)
    return out, res


def kernel(**inputs):
    in_maps, flags = _host_inputs(**inputs)
    out, _ = _run(in_maps, flags, trace=False)
    return out


# revision 34
# speedup vs baseline: 1.0001x; 1.0001x over previous
"""AttnBlock fusion kernel for Trainium2 (Bass/Tile), 8 NeuronCores.

Reference computation (per batch element b; c=512 channels, hw=1024 spatial):
    h  = GroupNorm(32, c)(x) ; k = Wk h + bk ; v = Wv h + bv
    y_ = GroupNorm(32, c)(y) ; q = Wq y_ + bq
    attn = softmax_j(q^T k / sqrt(c)) ; o = v @ attn^T ; out = x + Wp o + bp

Sharding: pure data parallel over batch (16 batches / 8 cores = 2 each).

Algebraic folds (host side, exact):
  * S = q^T k = y_^T (Wq^T Wk) h  -> A := Wq^T Wk precomputed; the q and k
    projections disappear (one matmul t = A h replaces both).
  * Wp (v @ P) = (Wp Wv) h @ P    -> Bm := Wp Wv precomputed; the v and
    proj_out projections disappear (u = Bm h replaces both).
  * bk adds a per-i constant to logits -> cancels in softmax.
  * bv contributes Wp bv exactly (softmax rows sum to 1) -> bp' = bp + Wp bv.
  * bq (zero in practice) handled by a compiled-in logit-bias path.

Precision scheme (validated vs reference: rel_l2 ~ 5.6e-3, gate 2e-2):
  * All five big matmuls run fp8(e4m3) with MatmulPerfMode.DoubleRow:
    2 contraction tiles per pass = 2x throughput over fp32r/bf16.
  * A, Bm scaled by 16 so t = A h and u = Bm h land in e4m3 range (+-240);
    1/16 is folded into the exp scale (t side) and into ones=16 for the
    Z row-sum matmul (u side, via 1/Z).
  * E = exp(s S - 3): the -3 shift is softmax-invariant and keeps
    max(E) ~ 31 < 240 so no fp8 overflow-to-inf.
  * x, y stored bf16 on chip (stats + gn + residual), accumulation fp32.

Engine split per batch (PE ~28us is the bound):
  PE      t/uT/S/Z/o DoubleRow matmuls + tiny GN aggregation matmul
  Scalar  exp(S), t PSUM->fp8 copies, GN sqrt
  DVE     bn_stats GN statistics, uT copies, 1/Z (fast approx), o*(1/Z)
  GpSimd  GN apply (bf16->fp8), residual add to output
"""

import math
import os
import sys
from contextlib import ExitStack

import numpy as np
import ml_dtypes

for _p in ("/opt/trn_rl_repo", "/root/.axon_site/_ro/trn_rl_repo"):
    if os.path.isdir(_p) and _p not in sys.path:
        sys.path.append(_p)

import concourse.bass as bass
import concourse.bacc as bacc
import concourse.mybir as mybir
import concourse.tile as tile
from concourse.bass_utils import run_bass_kernel_spmd

F32 = mybir.dt.float32
BF16 = mybir.dt.bfloat16
F8 = mybir.dt.float8e4
U8 = mybir.dt.uint8
U16 = mybir.dt.uint16
AF = mybir.ActivationFunctionType
ALU = mybir.AluOpType
DR = mybir.MatmulPerfMode.DoubleRow

B, C, H, W = 16, 512, 32, 32
HW = H * W                  # 1024
NCORES = 8
BPC = B // NCORES           # 2 batches per core
P = 128                     # SBUF partitions
CT = C // P                 # 4 channel tiles
JT = HW // P                # 8 key-position tiles
IBS = 512                   # query positions per i-block
IB = HW // IBS              # 2 i-blocks
GROUPS = 32
GSIZE = C // GROUPS         # 16 channels per group
EPS = 1e-6
SM = float(C) ** -0.5
SA = 16.0                   # scale folded into A
SB = 16.0                   # scale folded into Bm (and into ones for Z)
EXPS = SM / SA
EXPB = -3.0                 # softmax-invariant logit shift, keeps E < 240

NPF8 = ml_dtypes.float8_e4m3   # IEEE e4m3 (bias 7, max 240) == TRN FP8_EXP4
NPBF16 = ml_dtypes.bfloat16


def _emit(tc, aps, has_bq, has_bpp, id_aff):
    nc = tc.nc
    xs, ys, out = aps["xs"], aps["ys"], aps["out"]

    with ExitStack() as ctx:
        cpool = ctx.enter_context(tc.tile_pool(name="const", bufs=1))
        wpool = ctx.enter_context(tc.tile_pool(name="w", bufs=1))
        xpool = ctx.enter_context(tc.tile_pool(name="xin", bufs=2))
        ypool = ctx.enter_context(tc.tile_pool(name="yin", bufs=2))
        hpool = ctx.enter_context(tc.tile_pool(name="hb", bufs=2))
        ynpool = ctx.enter_context(tc.tile_pool(name="ynb", bufs=2))
        tpool = ctx.enter_context(tc.tile_pool(name="tb", bufs=2))
        upool = ctx.enter_context(tc.tile_pool(name="ub", bufs=2))
        epool = ctx.enter_context(tc.tile_pool(name="eb", bufs=2))
        stpool = ctx.enter_context(tc.tile_pool(name="st", bufs=2))
        smpool = ctx.enter_context(tc.tile_pool(name="sm", bufs=3))
        ompool = ctx.enter_context(tc.tile_pool(name="om", bufs=3))
        outpool = ctx.enter_context(tc.tile_pool(name="outb", bufs=2))
        rzpool = ctx.enter_context(tc.tile_pool(name="rz", bufs=2))
        # one 7-bank ring for S/t/uT tiles AND o accumulators: during the
        # S phase all 7 banks buffer S tiles so the PE can run far ahead of
        # the exp drain; o accumulators then claim 4 slots by rotation
        pspool = ctx.enter_context(tc.tile_pool(name="ps", bufs=7, space="PSUM"))
        zpool = ctx.enter_context(tc.tile_pool(name="z", bufs=1, space="PSUM"))
        opool = pspool

        # ---- constants / weights ----
        prm_sb = cpool.tile([P, 5, CT], F32)
        nc.gpsimd.dma_start(prm_sb[:], aps["prm"].rearrange("p (q t) -> p q t", t=CT))
        amat_sb = cpool.tile([P, P], F32)
        nc.gpsimd.dma_start(amat_sb[:], aps["amat"][:])
        ones_sb = cpool.tile([P, 2, P], F8)
        nc.vector.memset(ones_sb[:], SB)
        expb_sb = cpool.tile([P, 1], F32)
        nc.vector.memset(expb_sb[:], EXPB)
        eps_sb = cpool.tile([P, 1], F32)
        nc.vector.memset(eps_sb[:], EPS)
        # dummy matmul to absorb the PE's cold-start latency before the
        # first real (dependency-gated) matmul arrives
        wps = pspool.tile([P, P], F32, tag="ps", name="warm")
        nc.tensor.matmul(wps[:], ones_sb[:], ones_sb[:], start=True, stop=True,
                         perf_mode=DR)
        A_sb = wpool.tile([P, 2, 2, C], F8)
        nc.gpsimd.dma_start(
            A_sb[:], aps["A"].rearrange("p (a b o) -> p a b o", a=2, b=2).bitcast(F8)
        )
        Bm_sb = wpool.tile([P, 2, 2, C], F8)
        nc.gpsimd.dma_start(
            Bm_sb[:], aps["Bm"].rearrange("p (a b o) -> p a b o", a=2, b=2).bitcast(F8)
        )
        if has_bq:
            g_sb = cpool.tile([P, CT], F8)
            nc.gpsimd.dma_start(g_sb[:], aps["gv"].bitcast(F8))

        def load_xy(b):
            """Split DMAs so stats can start before the full tensor lands;
            x0 per-tile since its first tile gates the whole pipeline."""
            x_sb = xpool.tile([P, CT, HW], BF16, tag="x")
            v = xs[b].rearrange("p (t n) -> p t n", n=HW).bitcast(BF16)
            step = 1 if b == 0 else 2
            for c0 in range(0, CT, step):
                nc.sync.dma_start(x_sb[:, c0 : c0 + step, :],
                                  v[:, c0 : c0 + step, :])
            y_sb = ypool.tile([P, CT, HW], BF16, tag="y")
            v = ys[b].rearrange("p (t n) -> p t n", n=HW).bitcast(BF16)
            for hf in range(2):
                nc.sync.dma_start(y_sb[:, 2 * hf : 2 * hf + 2, :],
                                  v[:, 2 * hf : 2 * hf + 2, :])
            return x_sb, y_sb

        def stats_pre(src, st, u):
            """DVE bn_stats: per-channel [mean, var, mean^2] -> st[:, u]."""
            for t in range(CT):
                bns = smpool.tile([P, 2, 6], F32, tag="bns")
                for h2 in range(2):
                    nc.vector.bn_stats(
                        bns[:, h2, :], src[:, t, h2 * 512 : (h2 + 1) * 512]
                    )
                nc.vector.bn_aggr(st[:, u, t, 0:2], bns[:])
            nc.vector.tensor_tensor(
                st[:, u, :, 2], st[:, u, :, 0], st[:, u, :, 0], op=ALU.mult
            )

        def stats_mm(st, u):
            """PE: per-group averaging of [mean, var, mean^2] for one tensor."""
            gt = pspool.tile([P, IBS], F32, tag="ps", name="gps")
            gps = gt[:, 0 : CT * 3]
            nc.tensor.matmul(gps, amat_sb[:], st[:, u], start=True, stop=True)
            return gps

        def stats_post(gps, u, uid):
            """a = rstd*gamma, mb = beta - mean*a for one tensor: [P, CT].
            rstd = 1/(sqrt on ACT, accurate reciprocal on DVE); chain kept
            short because each serial hop pays a scheduler-interleave delay."""
            g = smpool.tile([P, CT, 3], F32, tag=f"g{uid}")
            nc.vector.tensor_copy(g[:], gps)
            # var_g = E[var] + E[mean^2] - E[mean]^2 (equal-count partitions);
            # mid-chain ops on GpSimd so DVE backfill can't delay the hops
            msq = smpool.tile([P, CT], F32, tag=f"ms{uid}")
            nc.gpsimd.tensor_tensor(msq[:], g[:, :, 0], g[:, :, 0], op=ALU.mult)
            var = smpool.tile([P, CT], F32, tag=f"va{uid}")
            nc.gpsimd.tensor_tensor(var[:], g[:, :, 1], g[:, :, 2], op=ALU.add)
            nc.gpsimd.tensor_tensor(var[:], var[:], msq[:], op=ALU.subtract)
            std = smpool.tile([P, CT], F32, tag=f"sd{uid}")
            nc.scalar.activation(std[:], var[:], AF.Sqrt, bias=eps_sb[:])
            r0 = smpool.tile([P, CT], F32, tag=f"r0{uid}")
            nc.vector.reciprocal(r0[:], std[:])
            mb = smpool.tile([P, CT], F32, tag=f"mb{uid}")
            if id_aff:
                # gamma == 1, beta == 0: a = rstd, mb = -mean*rstd
                nc.vector.scalar_tensor_tensor(
                    mb[:], g[:, :, 0], -1.0, r0[:], op0=ALU.mult, op1=ALU.mult
                )
                return r0, mb
            a = smpool.tile([P, CT], F32, tag=f"a{uid}")
            nc.gpsimd.tensor_tensor(a[:], r0[:], prm_sb[:, u, :], op=ALU.mult)
            nc.gpsimd.tensor_tensor(mb[:], g[:, :, 0], a[:], op=ALU.mult)
            nc.gpsimd.tensor_tensor(mb[:], prm_sb[:, 2 + u, :], mb[:], op=ALU.subtract)
            return a, mb

        def gn_apply(src, pool, tag, ab, engines):
            """Per-tile affine-normalize, tile t on engines[t] (bf16 -> fp8)."""
            a, mb = ab
            d = pool.tile([P, CT, HW], F8, tag=tag)
            for t in range(CT):
                eng = engines[t]
                if eng is nc.scalar:
                    nc.scalar.activation(
                        d[:, t, :], src[:, t, :], AF.Identity,
                        bias=mb[:, t : t + 1], scale=a[:, t : t + 1],
                    )
                else:
                    eng.tensor_scalar(
                        d[:, t, :], src[:, t, :], a[:, t : t + 1],
                        mb[:, t : t + 1], op0=ALU.mult, op1=ALU.add,
                    )
            return d

        V, G, SC = nc.vector, nc.gpsimd, nc.scalar

        def emit_t(h_sb):
            """t = A h  (t[cy, j], fp8; copies on Scalar)."""
            t_sb = tpool.tile([P, CT, HW], F8, tag="t", name="t")
            for nh in range(IB):
                for mt in range(CT):
                    ps = pspool.tile([P, IBS], F32, tag="ps", name="ps")
                    for kp in range(2):
                        nc.tensor.matmul(
                            ps[:],
                            A_sb[:, kp, :, mt * P : (mt + 1) * P],
                            h_sb[:, 2 * kp : 2 * kp + 2, nh * IBS : (nh + 1) * IBS],
                            start=(kp == 0), stop=(kp == 1), perf_mode=DR,
                        )
                    nc.scalar.copy(t_sb[:, mt, nh * IBS : (nh + 1) * IBS], ps[:])
            return t_sb

        def emit_uT(h_sb):
            """uT = h^T Bm^T  (uT[j, co], fp8; copies split Scalar/DVE)."""
            uT_sb = upool.tile([P, JT, C], F8, tag="u", name="u")
            for jt in range(JT):
                ps = pspool.tile([P, C], F32, tag="ps", name="ps")
                for kp in range(2):
                    nc.tensor.matmul(
                        ps[:],
                        h_sb[:, 2 * kp : 2 * kp + 2, jt * P : (jt + 1) * P],
                        Bm_sb[:, kp, :, :],
                        start=(kp == 0), stop=(kp == 1), perf_mode=DR,
                    )
                if jt < 4:
                    nc.scalar.copy(uT_sb[:, jt, :], ps[:])
                else:
                    nc.vector.tensor_copy(uT_sb[:, jt, :], ps[:])
            return uT_sb

        def emit_bias(h_sb):
            """bq logit bias: r[j] = g^T h, bias = SM*r + EXPB."""
            rps = zpool.tile([P, JT], F32, tag="z", name="rb")
            for jt in range(JT):
                for kt in range(CT):
                    nc.tensor.matmul(
                        rps[:, jt : jt + 1],
                        h_sb[:, kt, jt * P : (jt + 1) * P],
                        g_sb[:, kt : kt + 1],
                        start=(kt == 0), stop=(kt == CT - 1),
                    )
            bias_sb = smpool.tile([P, JT], F32, tag="bia", name="bia")
            nc.vector.tensor_scalar(
                bias_sb[:], rps[:], SM, EXPB, op0=ALU.mult, op1=ALU.add
            )
            return bias_sb

        def emit_attention(b, t_sb, uT_sb, yn_sb, xres, bias_sb, mid=None):
            outv = out[b].rearrange("p (t n) -> p t n", n=HW)
            e = [
                epool.tile([P, JT, IBS], F8, tag=f"e{ib}", name=f"e{ib}")
                for ib in range(IB)
            ]
            zps = {}

            def S_group(ib, jt):
                ps = pspool.tile([P, IBS], F32, tag="ps", name="ps")
                for kp in range(2):
                    nc.tensor.matmul(
                        ps[:],
                        t_sb[:, 2 * kp : 2 * kp + 2, jt * P : (jt + 1) * P],
                        yn_sb[:, 2 * kp : 2 * kp + 2, ib * IBS : (ib + 1) * IBS],
                        start=(kp == 0), stop=(kp == 1), perf_mode=DR,
                    )
                bias = bias_sb[:, jt : jt + 1] if has_bq else expb_sb[:]
                nc.scalar.activation(
                    e[ib][:, jt, :], ps[:], AF.Exp, bias=bias, scale=EXPS
                )

            def Z_mm(ib, pr):
                if pr == 0:
                    zps[ib] = zpool.tile([P, IBS], F32, tag="z", name="z")
                nc.tensor.matmul(
                    zps[ib][:], ones_sb[:],
                    e[ib][:, 2 * pr : 2 * pr + 2, :],
                    start=(pr == 0), stop=(pr == 3), perf_mode=DR,
                )

            def recip(ib):
                rz = rzpool.tile([P, IBS], F32, tag="rz", name="rz")
                with tc.high_priority():
                    nc.vector.reciprocal_approx_fast(rz[:], zps[ib][:])
                return rz

            def o_block(ib, rz, ot, last=False):
                isl = slice(ib * IBS, (ib + 1) * IBS)
                for ct in range(CT):
                    ops_ = opool.tile([P, IBS], F32, tag="ps", name="o")
                    for pr in range(4):
                        nc.tensor.matmul(
                            ops_[:],
                            uT_sb[:, 2 * pr : 2 * pr + 2, ct * P : (ct + 1) * P],
                            e[ib][:, 2 * pr : 2 * pr + 2, :],
                            start=(pr == 0), stop=(pr == 3), perf_mode=DR,
                        )
                    om = ompool.tile([P, IBS], F32, tag="om", name="om")
                    # high priority: PSUM drains must preempt lower-priority
                    # stats backfill in the DVE ready-queue or the o banks
                    # starve the next i-block's matmuls
                    with tc.high_priority():
                        nc.vector.tensor_tensor(om[:], ops_[:], rz[:], op=ALU.mult)
                        # final block: drain on DVE + per-ct DMA, shortest tail
                        eng = nc.vector if last else nc.gpsimd
                        eng.tensor_tensor(
                            ot[:, ct, :], om[:], xres[:, ct, isl], op=ALU.add
                        )
                        if last:
                            nc.sync.dma_start(outv[:, ct, isl], ot[:, ct, :])
                if not last:
                    nc.sync.dma_start(outv[:, :, isl], ot[:])

            ot0 = outpool.tile([P, CT, IBS], F32, tag="ot", name="ot0")
            ot1 = outpool.tile([P, CT, IBS], F32, tag="ot", name="ot1")
            for jt in range(JT):
                S_group(0, jt)
            for pr in range(3):
                Z_mm(0, pr)
            for jt in range(4):
                S_group(1, jt)
            Z_mm(0, 3)
            rz0 = recip(0)
            o_block(0, rz0, ot0)
            for jt in range(4, JT):
                S_group(1, jt)
            if mid is not None:
                # next-batch GN emitted here: priority below this batch's
                # exps/S-matmuls, above the o/Z tail — the scheduler slots it
                # into exp-wait gaps without blocking either batch
                mid()
            for pr in range(3):
                Z_mm(1, pr)
            Z_mm(1, 3)
            rz1 = recip(1)
            o_block(1, rz1, ot1, last=(b == BPC - 1))

        def make_xres(x_sb):
            if not has_bpp:
                return x_sb
            xres = outpool.tile([P, CT, HW], F32, tag="xb", name="xb")
            for t in range(CT):
                nc.gpsimd.tensor_scalar(
                    xres[:, t, :], x_sb[:, t, :], prm_sb[:, 4, t : t + 1],
                    None, op0=ALU.add,
                )
            return xres

        # ---- all GroupNorm work is front-loaded (BPC=2): the x0 chain gates
        # the first matmul; everything else fills scheduler bubbles. All
        # Sqrts precede the first Exp so the activation table loads only
        # twice in the whole kernel.
        xy = [load_xy(b) for b in range(BPC)]
        sts = [stpool.tile([P, 2, CT, 3], F32, tag="st", name=f"st{b}")
               for b in range(BPC)]

        stats_pre(xy[0][0], sts[0], 0)
        ab = stats_post(stats_mm(sts[0], 0), 0, "x0")
        h0 = gn_apply(xy[0][0], hpool, "h", ab, [V, V, SC, G])
        t0 = emit_t(h0)
        stats_pre(xy[0][1], sts[0], 1)
        ab = stats_post(stats_mm(sts[0], 1), 1, "y0")
        yn0 = gn_apply(xy[0][1], ynpool, "yn", ab, [V, SC, G, G])
        u0 = emit_uT(h0)
        bias0 = emit_bias(h0) if has_bq else None

        hs, yns, ts, us, biases = [h0], [yn0], [t0], [u0], [bias0]
        for b in range(BPC):
            xres = make_xres(xy[b][0])
            if b > 0:
                ts.append(emit_t(hs[b]))
                us.append(emit_uT(hs[b]))
                biases.append(emit_bias(hs[b]) if has_bq else None)
            def mid_fn(bn):
                stats_pre(xy[bn][0], sts[bn], 0)
                ab = stats_post(stats_mm(sts[bn], 0), 0, f"x{bn}")
                hs.append(gn_apply(xy[bn][0], hpool, "h", ab, [V, V, G, G]))
                stats_pre(xy[bn][1], sts[bn], 1)
                ab = stats_post(stats_mm(sts[bn], 1), 1, f"y{bn}")
                yns.append(gn_apply(xy[bn][1], ynpool, "yn", ab, [V, V, G, G]))

            mid = (lambda bn=b + 1: mid_fn(bn)) if b + 1 < BPC else None
            emit_attention(b, ts[b], us[b], yns[b], xres, biases[b], mid=mid)


_CACHE = {}


def _build(has_bq, has_bpp, id_aff):
    key = ("nc", has_bq, has_bpp, id_aff)
    if key in _CACHE:
        return _CACHE[key]
    nc = bacc.Bacc("TRN2", target_bir_lowering=False, debug=False)
    aps = {
        "xs": nc.dram_tensor("xs", [BPC, P, CT * HW], U16, kind="ExternalInput").ap(),
        "ys": nc.dram_tensor("ys", [BPC, P, CT * HW], U16, kind="ExternalInput").ap(),
        "A": nc.dram_tensor("A", [P, 4 * C], U8, kind="ExternalInput").ap(),
        "Bm": nc.dram_tensor("Bm", [P, 4 * C], U8, kind="ExternalInput").ap(),
        "prm": nc.dram_tensor("prm", [P, 5 * CT], F32, kind="ExternalInput").ap(),
        "amat": nc.dram_tensor("amat", [P, P], F32, kind="ExternalInput").ap(),
        "out": nc.dram_tensor("out", [BPC, P, CT * HW], F32, kind="ExternalOutput").ap(),
    }
    if has_bq:
        aps["gv"] = nc.dram_tensor("gv", [P, CT], U8, kind="ExternalInput").ap()
    with tile.TileContext(nc) as tc:
        _emit(tc, aps, has_bq, has_bpp, id_aff)
    nc.compile()
    _CACHE[key] = nc
    return nc


def _pack_chw(a):
    """[*, C, HW] -> [*, P, CT*HW] matching SBUF layout c = t*128 + p."""
    lead = a.shape[:-2]
    a = a.reshape(*lead, CT, P, HW)
    a = np.moveaxis(a, -3, -2)          # [..., P, CT, HW]
    return np.ascontiguousarray(a.reshape(*lead, P, CT * HW))


def _unpack_chw(a):
    """[*, P, CT*HW] -> [*, C, HW]."""
    lead = a.shape[:-2]
    a = a.reshape(*lead, P, CT, HW)
    a = np.moveaxis(a, -2, -3)          # [..., CT, P, HW]
    return np.ascontiguousarray(a.reshape(*lead, CT * P, HW))


def _q8(a):
    return np.clip(a, -240.0, 240.0).astype(NPF8)


def _pack_w(wT, scale):
    """wT [cin, cout] -> fp8 bytes [P, 2*2*C]: [p, kpair, ktile2, cout],
    cin = (2*kpair + ktile2)*128 + p."""
    w8 = _q8(wT * scale).view(np.uint8)
    w8 = w8.reshape(2, 2, P, C).transpose(2, 0, 1, 3)
    return np.ascontiguousarray(w8.reshape(P, 4 * C))


def _host_inputs(x, y, norm_scale, norm_bias, norm1_scale, norm1_bias,
                 wq, bq, wk, bk, wv, bv, wp, bp):
    f = lambda a: np.ascontiguousarray(np.asarray(a, dtype=np.float32))
    x = f(x).reshape(B, C, HW)
    y = f(y).reshape(B, C, HW)
    wq, wk, wv, wp = f(wq), f(wk), f(wv), f(wp)
    A = wq.T @ wk                       # [cy, ch]
    Bm = wp @ wv                        # [co, ci]
    # bk cancels in softmax; bv folds into bp' because softmax rows sum to 1
    bpp = f(bp) + wp @ f(bv)
    # rows: [gamma_x, gamma_y, beta_x, beta_y, bpp]
    prm = np.stack([f(norm_scale), f(norm1_scale), f(norm_bias), f(norm1_bias),
                    bpp]).astype(np.float32)
    prm = np.ascontiguousarray(
        prm.reshape(5, CT, P).transpose(2, 0, 1).reshape(P, 5 * CT)
    )
    amat = np.zeros((P, P), np.float32)
    for g in range(P // GSIZE):
        amat[g * GSIZE : (g + 1) * GSIZE, g * GSIZE : (g + 1) * GSIZE] = 1.0 / GSIZE
    has_bq = bool(np.any(np.asarray(bq)))
    has_bpp = bool(np.any(bpp))
    id_aff = bool(
        np.all(prm[:, 0 * CT : 2 * CT] == 1.0) and
        np.all(prm[:, 2 * CT : 4 * CT] == 0.0)
    )
    shared = {
        "A": _pack_w(A.T, SA),          # lhsT[cin=ch, cout=cy]
        "Bm": _pack_w(Bm.T, SB),        # rhs[cin=ci, cout=co]
        "prm": prm, "amat": amat,
    }
    if has_bq:
        gv = wk.T @ f(bq)               # [ci]
        gv8 = _q8(gv).view(np.uint8).reshape(CT, P).T
        shared["gv"] = np.ascontiguousarray(gv8)

    xb = _pack_chw(x.astype(NPBF16).view(np.uint16))
    yb = _pack_chw(y.astype(NPBF16).view(np.uint16))
    in_maps = []
    for core in range(NCORES):
        sl = slice(core * BPC, (core + 1) * BPC)
        in_maps.append({"xs": xb[sl], "ys": yb[sl], **shared})
    return in_maps, (has_bq, has_bpp, id_aff)


def _run(in_maps, flags, trace=False):
    nc = _build(*flags)
    res = run_bass_kernel_spmd(
        nc, in_maps, core_ids=list(range(NCORES)), trace=trace
    )
    out = np.concatenate(
        [_unpack_chw(res.results[i]["out"]) for i in range(NCORES)], axis=0
    ).reshape(B, C, H, W)
    return out, res


def kernel(**inputs):
    in_maps, flags = _host_inputs(**inputs)
    out, _ = _run(in_maps, flags, trace=False)
    return out


# revision 40
# speedup vs baseline: 1.0565x; 1.0564x over previous
"""AttnBlock fusion kernel for Trainium2 (Bass/Tile), 8 NeuronCores.

Reference computation (per batch element b; c=512 channels, hw=1024 spatial):
    h  = GroupNorm(32, c)(x) ; k = Wk h + bk ; v = Wv h + bv
    y_ = GroupNorm(32, c)(y) ; q = Wq y_ + bq
    attn = softmax_j(q^T k / sqrt(c)) ; o = v @ attn^T ; out = x + Wp o + bp

Sharding: pure data parallel over batch (16 batches / 8 cores = 2 each).

Algebraic folds (host side, exact):
  * S = q^T k = y_^T (Wq^T Wk) h  -> A := Wq^T Wk precomputed; the q and k
    projections disappear (one matmul t = A h replaces both).
  * Wp (v @ P) = (Wp Wv) h @ P    -> Bm := Wp Wv precomputed; the v and
    proj_out projections disappear (u = Bm h replaces both).
  * bk adds a per-i constant to logits -> cancels in softmax.
  * bv contributes Wp bv exactly (softmax rows sum to 1) -> bp' = bp + Wp bv.
  * bq (zero in practice) handled by a compiled-in logit-bias path.

Precision scheme (validated vs reference: rel_l2 ~ 5.6e-3, gate 2e-2):
  * All five big matmuls run fp8(e4m3) with MatmulPerfMode.DoubleRow:
    2 contraction tiles per pass = 2x throughput over fp32r/bf16.
  * A, Bm scaled by 16 so t = A h and u = Bm h land in e4m3 range (+-240);
    1/16 is folded into the exp scale (t side) and into ones=16 for the
    Z row-sum matmul (u side, via 1/Z).
  * E = exp(s S - 3): the -3 shift is softmax-invariant and keeps
    max(E) ~ 31 < 240 so no fp8 overflow-to-inf.
  * x, y stored bf16 on chip (stats + gn + residual), accumulation fp32.

Engine split per batch (PE ~28us is the bound):
  PE      t/uT/S/Z/o DoubleRow matmuls + tiny GN aggregation matmul
  Scalar  exp(S), t PSUM->fp8 copies, GN sqrt
  DVE     bn_stats GN statistics, uT copies, 1/Z (fast approx), o*(1/Z)
  GpSimd  GN apply (bf16->fp8), residual add to output
"""

import math
import os
import sys
from contextlib import ExitStack, nullcontext

import numpy as np
import ml_dtypes

for _p in ("/opt/trn_rl_repo", "/root/.axon_site/_ro/trn_rl_repo"):
    if os.path.isdir(_p) and _p not in sys.path:
        sys.path.append(_p)

import concourse.bass as bass
import concourse.bacc as bacc
import concourse.mybir as mybir
import concourse.tile as tile
from concourse.bass_utils import run_bass_kernel_spmd

F32 = mybir.dt.float32
BF16 = mybir.dt.bfloat16
F8 = mybir.dt.float8e4
U8 = mybir.dt.uint8
U16 = mybir.dt.uint16
AF = mybir.ActivationFunctionType
ALU = mybir.AluOpType
DR = mybir.MatmulPerfMode.DoubleRow

B, C, H, W = 16, 512, 32, 32
HW = H * W                  # 1024
NCORES = 8
BPC = B // NCORES           # 2 batches per core
P = 128                     # SBUF partitions
CT = C // P                 # 4 channel tiles
JT = HW // P                # 8 key-position tiles
IBS = 512                   # query positions per i-block
IB = HW // IBS              # 2 i-blocks
GROUPS = 32
GSIZE = C // GROUPS         # 16 channels per group
EPS = 1e-6
SM = float(C) ** -0.5
SA = 16.0                   # scale folded into A
SB = 16.0                   # scale folded into Bm (and into ones for Z)
EXPS = SM / SA
EXPB = -3.0                 # softmax-invariant logit shift, keeps E < 240

NPF8 = ml_dtypes.float8_e4m3   # IEEE e4m3 (bias 7, max 240) == TRN FP8_EXP4
NPBF16 = ml_dtypes.bfloat16


def _emit(tc, aps, has_bq, has_bpp, id_aff):
    nc = tc.nc
    xs, ys, out = aps["xs"], aps["ys"], aps["out"]

    with ExitStack() as ctx:
        cpool = ctx.enter_context(tc.tile_pool(name="const", bufs=1))
        wpool = ctx.enter_context(tc.tile_pool(name="w", bufs=1))
        xpool = ctx.enter_context(tc.tile_pool(name="xin", bufs=2))
        ypool = ctx.enter_context(tc.tile_pool(name="yin", bufs=2))
        hpool = ctx.enter_context(tc.tile_pool(name="hb", bufs=2))
        ynpool = ctx.enter_context(tc.tile_pool(name="ynb", bufs=2))
        tpool = ctx.enter_context(tc.tile_pool(name="tb", bufs=2))
        upool = ctx.enter_context(tc.tile_pool(name="ub", bufs=2))
        epool = ctx.enter_context(tc.tile_pool(name="eb", bufs=2))
        stpool = ctx.enter_context(tc.tile_pool(name="st", bufs=2))
        smpool = ctx.enter_context(tc.tile_pool(name="sm", bufs=3))
        ompool = ctx.enter_context(tc.tile_pool(name="om", bufs=3))
        outpool = ctx.enter_context(tc.tile_pool(name="outb", bufs=2))
        rzpool = ctx.enter_context(tc.tile_pool(name="rz", bufs=2))
        # one 7-bank ring for S/t/uT tiles AND o accumulators: during the
        # S phase all 7 banks buffer S tiles so the PE can run far ahead of
        # the exp drain; o accumulators then claim 4 slots by rotation
        pspool = ctx.enter_context(tc.tile_pool(name="ps", bufs=7, space="PSUM"))
        zpool = ctx.enter_context(tc.tile_pool(name="z", bufs=1, space="PSUM"))
        opool = pspool

        # ---- constants / weights ----
        prm_sb = cpool.tile([P, 5, CT], F32)
        nc.gpsimd.dma_start(prm_sb[:], aps["prm"].rearrange("p (q t) -> p q t", t=CT))
        amat_sb = cpool.tile([P, P], F32)
        nc.gpsimd.dma_start(amat_sb[:], aps["amat"][:])
        ones_sb = cpool.tile([P, 2, P], F8)
        nc.vector.memset(ones_sb[:], SB)
        expb_sb = cpool.tile([P, 1], F32)
        nc.vector.memset(expb_sb[:], EXPB)
        eps_sb = cpool.tile([P, 1], F32)
        nc.vector.memset(eps_sb[:], EPS)
        # dummy matmul to absorb the PE's cold-start latency before the
        # first real (dependency-gated) matmul arrives
        wps = pspool.tile([P, P], F32, tag="ps", name="warm")
        nc.tensor.matmul(wps[:], ones_sb[:], ones_sb[:], start=True, stop=True,
                         perf_mode=DR)
        A_sb = wpool.tile([P, 2, 2, C], F8)
        nc.gpsimd.dma_start(
            A_sb[:], aps["A"].rearrange("p (a b o) -> p a b o", a=2, b=2).bitcast(F8)
        )
        Bm_sb = wpool.tile([P, 2, 2, C], F8)
        nc.gpsimd.dma_start(
            Bm_sb[:], aps["Bm"].rearrange("p (a b o) -> p a b o", a=2, b=2).bitcast(F8)
        )
        if has_bq:
            g_sb = cpool.tile([P, CT], F8)
            nc.gpsimd.dma_start(g_sb[:], aps["gv"].bitcast(F8))

        def load_xy(b):
            """Split DMAs so stats can start before the full tensor lands;
            x0 per-tile since its first tile gates the whole pipeline."""
            x_sb = xpool.tile([P, CT, HW], BF16, tag="x")
            v = xs[b].rearrange("p (t n) -> p t n", n=HW).bitcast(BF16)
            step = 1 if b == 0 else 2
            for c0 in range(0, CT, step):
                nc.sync.dma_start(x_sb[:, c0 : c0 + step, :],
                                  v[:, c0 : c0 + step, :])
            y_sb = ypool.tile([P, CT, HW], BF16, tag="y")
            v = ys[b].rearrange("p (t n) -> p t n", n=HW).bitcast(BF16)
            for hf in range(2):
                nc.sync.dma_start(y_sb[:, 2 * hf : 2 * hf + 2, :],
                                  v[:, 2 * hf : 2 * hf + 2, :])
            return x_sb, y_sb

        def stats_pre(src, st, u):
            """DVE bn_stats: per-channel [mean, var, mean^2] -> st[:, u]."""
            for t in range(CT):
                bns = smpool.tile([P, 2, 6], F32, tag="bns")
                for h2 in range(2):
                    nc.vector.bn_stats(
                        bns[:, h2, :], src[:, t, h2 * 512 : (h2 + 1) * 512]
                    )
                nc.vector.bn_aggr(st[:, u, t, 0:2], bns[:])
            nc.vector.tensor_tensor(
                st[:, u, :, 2], st[:, u, :, 0], st[:, u, :, 0], op=ALU.mult
            )

        def stats_mm(st, u, hp=False):
            """PE: per-group averaging of [mean, var, mean^2] for one tensor."""
            gt = pspool.tile([P, IBS], F32, tag="ps", name="gps")
            gps = gt[:, 0 : CT * 3]
            with tc.high_priority() if hp else nullcontext():
                nc.tensor.matmul(gps, amat_sb[:], st[:, u], start=True, stop=True)
            return gps

        def stats_post(gps, u, uid, hp=False):
            hpc = lambda: tc.high_priority() if hp else nullcontext()
            """a = rstd*gamma, mb = beta - mean*a for one tensor: [P, CT].
            rstd = 1/(sqrt on ACT, accurate reciprocal on DVE); chain kept
            short because each serial hop pays a scheduler-interleave delay."""
            g = smpool.tile([P, CT, 3], F32, tag=f"g{uid}")
            with hpc():
                nc.vector.tensor_copy(g[:], gps)
            # var_g = E[var] + E[mean^2] - E[mean]^2 (equal-count partitions);
            # mid-chain ops on GpSimd so DVE backfill can't delay the hops
            msq = smpool.tile([P, CT], F32, tag=f"ms{uid}")
            var = smpool.tile([P, CT], F32, tag=f"va{uid}")
            with hpc():
                nc.gpsimd.tensor_tensor(msq[:], g[:, :, 0], g[:, :, 0], op=ALU.mult)
                nc.gpsimd.tensor_tensor(var[:], g[:, :, 1], g[:, :, 2], op=ALU.add)
                nc.gpsimd.tensor_tensor(var[:], var[:], msq[:], op=ALU.subtract)
            std = smpool.tile([P, CT], F32, tag=f"sd{uid}")
            # Sqrt stays at natural priority: hoisting it would interleave
            # Sqrt<->Exp activation-table loads into the exp stream
            nc.scalar.activation(std[:], var[:], AF.Sqrt, bias=eps_sb[:])
            r0 = smpool.tile([P, CT], F32, tag=f"r0{uid}")
            mb = smpool.tile([P, CT], F32, tag=f"mb{uid}")
            with hpc():
                nc.vector.reciprocal(r0[:], std[:])
                if id_aff:
                    # gamma == 1, beta == 0: a = rstd, mb = -mean*rstd
                    nc.vector.scalar_tensor_tensor(
                        mb[:], g[:, :, 0], -1.0, r0[:], op0=ALU.mult, op1=ALU.mult
                    )
                    return r0, mb
                a = smpool.tile([P, CT], F32, tag=f"a{uid}")
                nc.gpsimd.tensor_tensor(a[:], r0[:], prm_sb[:, u, :], op=ALU.mult)
                nc.gpsimd.tensor_tensor(mb[:], g[:, :, 0], a[:], op=ALU.mult)
                nc.gpsimd.tensor_tensor(
                    mb[:], prm_sb[:, 2 + u, :], mb[:], op=ALU.subtract
                )
            return a, mb

        def gn_apply(src, pool, tag, ab, engines, hp=False):
            """Affine-normalize in 8 half-tile units spread over engines
            (bf16 -> fp8); halves the wall time vs per-tile assignment."""
            a, mb = ab
            d = pool.tile([P, CT, HW], F8, tag=tag)
            for i, eng in enumerate(engines):
                t, hh = divmod(i, 2)
                sl = slice(hh * 512, (hh + 1) * 512)
                with tc.high_priority() if hp else nullcontext():
                    if eng is nc.scalar:
                        nc.scalar.activation(
                            d[:, t, sl], src[:, t, sl], AF.Identity,
                            bias=mb[:, t : t + 1], scale=a[:, t : t + 1],
                        )
                    else:
                        eng.tensor_scalar(
                            d[:, t, sl], src[:, t, sl], a[:, t : t + 1],
                            mb[:, t : t + 1], op0=ALU.mult, op1=ALU.add,
                        )
            return d

        V, G, SC = nc.vector, nc.gpsimd, nc.scalar

        def emit_t(h_sb):
            """t = A h  (t[cy, j], fp8; copies on Scalar)."""
            t_sb = tpool.tile([P, CT, HW], F8, tag="t", name="t")
            for nh in range(IB):
                for mt in range(CT):
                    ps = pspool.tile([P, IBS], F32, tag="ps", name="ps")
                    for kp in range(2):
                        nc.tensor.matmul(
                            ps[:],
                            A_sb[:, kp, :, mt * P : (mt + 1) * P],
                            h_sb[:, 2 * kp : 2 * kp + 2, nh * IBS : (nh + 1) * IBS],
                            start=(kp == 0), stop=(kp == 1), perf_mode=DR,
                        )
                    nc.scalar.copy(t_sb[:, mt, nh * IBS : (nh + 1) * IBS], ps[:])
            return t_sb

        def emit_uT(h_sb):
            """uT = h^T Bm^T  (uT[j, co], fp8; copies split Scalar/DVE)."""
            uT_sb = upool.tile([P, JT, C], F8, tag="u", name="u")
            for jt in range(JT):
                ps = pspool.tile([P, C], F32, tag="ps", name="ps")
                for kp in range(2):
                    nc.tensor.matmul(
                        ps[:],
                        h_sb[:, 2 * kp : 2 * kp + 2, jt * P : (jt + 1) * P],
                        Bm_sb[:, kp, :, :],
                        start=(kp == 0), stop=(kp == 1), perf_mode=DR,
                    )
                if jt < 4:
                    nc.scalar.copy(uT_sb[:, jt, :], ps[:])
                else:
                    nc.vector.tensor_copy(uT_sb[:, jt, :], ps[:])
            return uT_sb

        def emit_bias(h_sb):
            """bq logit bias: r[j] = g^T h, bias = SM*r + EXPB."""
            rps = zpool.tile([P, JT], F32, tag="z", name="rb")
            for jt in range(JT):
                for kt in range(CT):
                    nc.tensor.matmul(
                        rps[:, jt : jt + 1],
                        h_sb[:, kt, jt * P : (jt + 1) * P],
                        g_sb[:, kt : kt + 1],
                        start=(kt == 0), stop=(kt == CT - 1),
                    )
            bias_sb = smpool.tile([P, JT], F32, tag="bia", name="bia")
            nc.vector.tensor_scalar(
                bias_sb[:], rps[:], SM, EXPB, op0=ALU.mult, op1=ALU.add
            )
            return bias_sb

        def emit_attention(b, t_sb, uT_sb, yn_sb, xres, bias_sb, mid=None):
            outv = out[b].rearrange("p (t n) -> p t n", n=HW)
            e = [
                epool.tile([P, JT, IBS], F8, tag=f"e{ib}", name=f"e{ib}")
                for ib in range(IB)
            ]
            zps = {}

            def S_group(ib, jt):
                ps = pspool.tile([P, IBS], F32, tag="ps", name="ps")
                for kp in range(2):
                    nc.tensor.matmul(
                        ps[:],
                        t_sb[:, 2 * kp : 2 * kp + 2, jt * P : (jt + 1) * P],
                        yn_sb[:, 2 * kp : 2 * kp + 2, ib * IBS : (ib + 1) * IBS],
                        start=(kp == 0), stop=(kp == 1), perf_mode=DR,
                    )
                bias = bias_sb[:, jt : jt + 1] if has_bq else expb_sb[:]
                nc.scalar.activation(
                    e[ib][:, jt, :], ps[:], AF.Exp, bias=bias, scale=EXPS
                )

            def Z_mm(ib, pr):
                if pr == 0:
                    zps[ib] = zpool.tile([P, IBS], F32, tag="z", name="z")
                nc.tensor.matmul(
                    zps[ib][:], ones_sb[:],
                    e[ib][:, 2 * pr : 2 * pr + 2, :],
                    start=(pr == 0), stop=(pr == 3), perf_mode=DR,
                )

            def recip(ib):
                rz = rzpool.tile([P, IBS], F32, tag="rz", name="rz")
                with tc.high_priority():
                    nc.vector.reciprocal_approx_fast(rz[:], zps[ib][:])
                return rz

            def o_block(ib, rz, ot, last=False):
                isl = slice(ib * IBS, (ib + 1) * IBS)
                for ct in range(CT):
                    ops_ = opool.tile([P, IBS], F32, tag="ps", name="o")
                    for pr in range(4):
                        nc.tensor.matmul(
                            ops_[:],
                            uT_sb[:, 2 * pr : 2 * pr + 2, ct * P : (ct + 1) * P],
                            e[ib][:, 2 * pr : 2 * pr + 2, :],
                            start=(pr == 0), stop=(pr == 3), perf_mode=DR,
                        )
                    om = ompool.tile([P, IBS], F32, tag="om", name="om")
                    # high priority: PSUM drains must preempt lower-priority
                    # stats backfill in the DVE ready-queue or the o banks
                    # starve the next i-block's matmuls
                    with tc.high_priority():
                        nc.vector.tensor_tensor(om[:], ops_[:], rz[:], op=ALU.mult)
                        # final block: drain on DVE + per-ct DMA, shortest tail
                        eng = nc.vector if last else nc.gpsimd
                        eng.tensor_tensor(
                            ot[:, ct, :], om[:], xres[:, ct, isl], op=ALU.add
                        )
                        if last:
                            nc.sync.dma_start(outv[:, ct, isl], ot[:, ct, :])
                if not last:
                    nc.sync.dma_start(outv[:, :, isl], ot[:])

            ot0 = outpool.tile([P, CT, IBS], F32, tag="ot", name="ot0")
            ot1 = outpool.tile([P, CT, IBS], F32, tag="ot", name="ot1")
            for jt in range(JT):
                S_group(0, jt)
            for pr in range(3):
                Z_mm(0, pr)
            for jt in range(4):
                S_group(1, jt)
            Z_mm(0, 3)
            rz0 = recip(0)
            o_block(0, rz0, ot0)
            for jt in range(4, JT):
                S_group(1, jt)
            if mid is not None:
                # next-batch GN emitted here: priority below this batch's
                # exps/S-matmuls, above the o/Z tail — the scheduler slots it
                # into exp-wait gaps without blocking either batch
                mid()
            for pr in range(3):
                Z_mm(1, pr)
            Z_mm(1, 3)
            rz1 = recip(1)
            o_block(1, rz1, ot1, last=(b == BPC - 1))

        def make_xres(x_sb):
            if not has_bpp:
                return x_sb
            xres = outpool.tile([P, CT, HW], F32, tag="xb", name="xb")
            for t in range(CT):
                nc.gpsimd.tensor_scalar(
                    xres[:, t, :], x_sb[:, t, :], prm_sb[:, 4, t : t + 1],
                    None, op0=ALU.add,
                )
            return xres

        # ---- all GroupNorm work is front-loaded (BPC=2): the x0 chain gates
        # the first matmul; everything else fills scheduler bubbles. All
        # Sqrts precede the first Exp so the activation table loads only
        # twice in the whole kernel.
        xy = [load_xy(b) for b in range(BPC)]
        sts = [stpool.tile([P, 2, CT, 3], F32, tag="st", name=f"st{b}")
               for b in range(BPC)]

        E8 = [V, SC, G, V, SC, G, V, SC]
        stats_pre(xy[0][0], sts[0], 0)
        ab = stats_post(stats_mm(sts[0], 0), 0, "x0")
        h0 = gn_apply(xy[0][0], hpool, "h", ab, E8)
        t0 = emit_t(h0)
        stats_pre(xy[0][1], sts[0], 1)
        ab = stats_post(stats_mm(sts[0], 1), 1, "y0")
        yn0 = gn_apply(xy[0][1], ynpool, "yn", ab, E8)
        u0 = emit_uT(h0)
        bias0 = emit_bias(h0) if has_bq else None

        hs, yns, ts, us, biases = [h0], [yn0], [t0], [u0], [bias0]
        for b in range(BPC):
            xres = make_xres(xy[b][0])
            if b > 0:
                ts.append(emit_t(hs[b]))
                us.append(emit_uT(hs[b]))
                biases.append(emit_bias(hs[b]) if has_bq else None)
            def mid_fn(bn):
                stats_pre(xy[bn][0], sts[bn], 0)
                ab = stats_post(stats_mm(sts[bn], 0, hp=True), 0, f"x{bn}",
                                hp=True)
                hs.append(gn_apply(xy[bn][0], hpool, "h", ab, E8, hp=True))
                stats_pre(xy[bn][1], sts[bn], 1)
                ab = stats_post(stats_mm(sts[bn], 1, hp=True), 1, f"y{bn}",
                                hp=True)
                yns.append(gn_apply(xy[bn][1], ynpool, "yn", ab, E8, hp=True))

            mid = (lambda bn=b + 1: mid_fn(bn)) if b + 1 < BPC else None
            emit_attention(b, ts[b], us[b], yns[b], xres, biases[b], mid=mid)


_CACHE = {}


def _build(has_bq, has_bpp, id_aff):
    key = ("nc", has_bq, has_bpp, id_aff)
    if key in _CACHE:
        return _CACHE[key]
    nc = bacc.Bacc("TRN2", target_bir_lowering=False, debug=False)
    aps = {
        "xs": nc.dram_tensor("xs", [BPC, P, CT * HW], U16, kind="ExternalInput").ap(),
        "ys": nc.dram_tensor("ys", [BPC, P, CT * HW], U16, kind="ExternalInput").ap(),
        "A": nc.dram_tensor("A", [P, 4 * C], U8, kind="ExternalInput").ap(),
        "Bm": nc.dram_tensor("Bm", [P, 4 * C], U8, kind="ExternalInput").ap(),
        "prm": nc.dram_tensor("prm", [P, 5 * CT], F32, kind="ExternalInput").ap(),
        "amat": nc.dram_tensor("amat", [P, P], F32, kind="ExternalInput").ap(),
        "out": nc.dram_tensor("out", [BPC, P, CT * HW], F32, kind="ExternalOutput").ap(),
    }
    if has_bq:
        aps["gv"] = nc.dram_tensor("gv", [P, CT], U8, kind="ExternalInput").ap()
    with tile.TileContext(nc) as tc:
        _emit(tc, aps, has_bq, has_bpp, id_aff)
    nc.compile()
    _CACHE[key] = nc
    return nc


def _pack_chw(a):
    """[*, C, HW] -> [*, P, CT*HW] matching SBUF layout c = t*128 + p."""
    lead = a.shape[:-2]
    a = a.reshape(*lead, CT, P, HW)
    a = np.moveaxis(a, -3, -2)          # [..., P, CT, HW]
    return np.ascontiguousarray(a.reshape(*lead, P, CT * HW))


def _unpack_chw(a):
    """[*, P, CT*HW] -> [*, C, HW]."""
    lead = a.shape[:-2]
    a = a.reshape(*lead, P, CT, HW)
    a = np.moveaxis(a, -2, -3)          # [..., CT, P, HW]
    return np.ascontiguousarray(a.reshape(*lead, CT * P, HW))


def _q8(a):
    return np.clip(a, -240.0, 240.0).astype(NPF8)


def _pack_w(wT, scale):
    """wT [cin, cout] -> fp8 bytes [P, 2*2*C]: [p, kpair, ktile2, cout],
    cin = (2*kpair + ktile2)*128 + p."""
    w8 = _q8(wT * scale).view(np.uint8)
    w8 = w8.reshape(2, 2, P, C).transpose(2, 0, 1, 3)
    return np.ascontiguousarray(w8.reshape(P, 4 * C))


def _host_inputs(x, y, norm_scale, norm_bias, norm1_scale, norm1_bias,
                 wq, bq, wk, bk, wv, bv, wp, bp):
    f = lambda a: np.ascontiguousarray(np.asarray(a, dtype=np.float32))
    x = f(x).reshape(B, C, HW)
    y = f(y).reshape(B, C, HW)
    wq, wk, wv, wp = f(wq), f(wk), f(wv), f(wp)
    A = wq.T @ wk                       # [cy, ch]
    Bm = wp @ wv                        # [co, ci]
    # bk cancels in softmax; bv folds into bp' because softmax rows sum to 1
    bpp = f(bp) + wp @ f(bv)
    # rows: [gamma_x, gamma_y, beta_x, beta_y, bpp]
    prm = np.stack([f(norm_scale), f(norm1_scale), f(norm_bias), f(norm1_bias),
                    bpp]).astype(np.float32)
    prm = np.ascontiguousarray(
        prm.reshape(5, CT, P).transpose(2, 0, 1).reshape(P, 5 * CT)
    )
    amat = np.zeros((P, P), np.float32)
    for g in range(P // GSIZE):
        amat[g * GSIZE : (g + 1) * GSIZE, g * GSIZE : (g + 1) * GSIZE] = 1.0 / GSIZE
    has_bq = bool(np.any(np.asarray(bq)))
    has_bpp = bool(np.any(bpp))
    id_aff = bool(
        np.all(prm[:, 0 * CT : 2 * CT] == 1.0) and
        np.all(prm[:, 2 * CT : 4 * CT] == 0.0)
    )
    shared = {
        "A": _pack_w(A.T, SA),          # lhsT[cin=ch, cout=cy]
        "Bm": _pack_w(Bm.T, SB),        # rhs[cin=ci, cout=co]
        "prm": prm, "amat": amat,
    }
    if has_bq:
        gv = wk.T @ f(bq)               # [ci]
        gv8 = _q8(gv).view(np.uint8).reshape(CT, P).T
        shared["gv"] = np.ascontiguousarray(gv8)

    xb = _pack_chw(x.astype(NPBF16).view(np.uint16))
    yb = _pack_chw(y.astype(NPBF16).view(np.uint16))
    in_maps = []
    for core in range(NCORES):
        sl = slice(core * BPC, (core + 1) * BPC)
        in_maps.append({"xs": xb[sl], "ys": yb[sl], **shared})
    return in_maps, (has_bq, has_bpp, id_aff)


def _run(in_maps, flags, trace=False):
    nc = _build(*flags)
    res = run_bass_kernel_spmd(
        nc, in_maps, core_ids=list(range(NCORES)), trace=trace
    )
    out = np.concatenate(
        [_unpack_chw(res.results[i]["out"]) for i in range(NCORES)], axis=0
    ).reshape(B, C, H, W)
    return out, res


def kernel(**inputs):
    in_maps, flags = _host_inputs(**inputs)
    out, _ = _run(in_maps, flags, trace=False)
    return out
